# revision 15
# baseline (speedup 1.0000x reference)
"""Trainium2 Bass kernel for ChannelAttentionModel (segment avg/max -> tiny MLP ->
sigmoid gate -> per-point scale), SPMD across 8 NeuronCores.

Sharding: batch_ids is sorted with B=16 segments; core k owns batches 2k and
2k+1 (whole batches per device). Each batch range is padded to a fixed R points
by replicating the first row of the batch (max-safe); the extra rows' sum
contribution is subtracted via a host-computed correction term.

x is shipped and processed in bf16 (tolerance is 2e-2; bf16 keeps worst-case
elementwise error well under 1%), halving both host<->device transfer bytes and
on-device HBM traffic. Stats (sum via PE matmul with a ones vector into f32
PSUM, max via DVE) and the tiny MLP stay in f32.
"""

import sys

for _p in ("/opt/trn_rl_repo", "/root/.axon_site/_ro/trn_rl_repo"):
    if _p not in sys.path:
        sys.path.append(_p)

import numpy as np
import ml_dtypes

import concourse.bacc as bacc
import concourse.tile as tile
from concourse import bass, mybir
from concourse.bass_utils import run_bass_kernel_spmd
from concourse.masks import make_identity

NCORES = 8
B = 16
C = 64
H = 32
RPC = 2  # batch ranges per core
TP = 4096  # points per tile
FA = TP // 128  # free-dim point groups per partition (32)
F = FA * C  # free elems per partition per tile (2048)
DT = mybir.dt.float32
DTX = mybir.dt.bfloat16
NPX = ml_dtypes.bfloat16


def build_nc(R: int, chunk_tiles: int = 4, xbufs: int = 3):
    nc = bacc.Bacc("TRN2", target_bir_lowering=False, debug=False,
                   num_devices=NCORES, enable_asserts=False)

    xs = nc.dram_tensor("xs", [RPC, R, C], DTX, kind="ExternalInput")
    corrt = nc.dram_tensor("corrt", [C, RPC], DT, kind="ExternalInput")
    invct = nc.dram_tensor("invct", [C, RPC], DT, kind="ExternalInput")
    w1t = nc.dram_tensor("w1t", [C, H], DT, kind="ExternalInput")
    b1c = nc.dram_tensor("b1c", [H, 1], DT, kind="ExternalInput")
    w2t = nc.dram_tensor("w2t", [H, C], DT, kind="ExternalInput")
    b2x2 = nc.dram_tensor("b2x2", [C, 1], DT, kind="ExternalInput")
    out = nc.dram_tensor("out", [RPC, R, C], DTX, kind="ExternalOutput")

    def dram_chunk_ap(handle, r, off, npts):
        return handle.ap()[r, off:off + npts, :].rearrange(
            "(p a) c -> p (a c)", p=128)

    # chunk each range into large DMA transfers
    chunks = []
    off = 0
    while off < R:
        npts = min(chunk_tiles * TP, R - off)
        chunks.append((off, npts))
        off += npts

    FCMAX = chunk_tiles * F

    # Chunks that stay resident in SBUF between phase 1 and phase 2 (skips
    # their phase-2 re-read from HBM): as many of range 0's chunks as fit,
    # plus range 1's last chunk. Budget: <=136 KiB/partition.
    res_budget = 136 * 1024
    resident = set()
    used = 0
    for ci, (off, npts) in enumerate(chunks):
        fc_b = npts * C // 128 * 2  # bf16 bytes per partition
        if used + fc_b > res_budget:
            break
        used += fc_b
        resident.add((0, ci))
    last = len(chunks) - 1
    last_b = chunks[last][1] * C // 128 * 2
    if used + last_b <= res_budget:
        used += last_b
        resident.add((1, last))

    with tile.TileContext(nc) as tc:
        with (
            tc.tile_pool(name="const", bufs=1) as const,
            tc.tile_pool(name="resp", bufs=1) as resp,
            tc.tile_pool(name="xpool", bufs=xbufs) as xpool,
            tc.tile_pool(name="accs", bufs=1) as accs,
            tc.tile_pool(name="small", bufs=1) as small,
            tc.tile_pool(name="psum_t", bufs=1, space="PSUM") as psum_t,
            tc.tile_pool(name="psum_w", bufs=1, space="PSUM") as psum_w,
        ):
            # constants
            ident = const.tile([128, 128], DT)
            make_identity(nc, ident[:])
            ones_row = const.tile([1, 128], DT)
            nc.vector.memset(ones_row[:], 1.0)
            ones_bf = const.tile([128, 1], mybir.dt.bfloat16)
            nc.vector.memset(ones_bf[:], 1.0)
            # const loads go on the scalar ring so the first x-chunk loads
            # (sync ring, FIFO) start immediately
            corrt_sb = const.tile([C, RPC], DT)
            nc.scalar.dma_start(out=corrt_sb[:], in_=corrt.ap())
            invct_sb = const.tile([C, RPC], DT)
            nc.scalar.dma_start(out=invct_sb[:], in_=invct.ap())
            w1t_sb = const.tile([C, H], DT)
            nc.scalar.dma_start(out=w1t_sb[:], in_=w1t.ap())
            b1c_sb = const.tile([H, 1], DT)
            nc.scalar.dma_start(out=b1c_sb[:], in_=b1c.ap())
            w2t_sb = const.tile([H, C], DT)
            nc.scalar.dma_start(out=w2t_sb[:], in_=w2t.ap())
            b2x2_sb = const.tile([C, 1], DT)
            nc.scalar.dma_start(out=b2x2_sb[:], in_=b2x2.ap())

            # Per-range pipeline: range 0 streams in (chunks stay resident
            # in SBUF), its fold+MLP+scale runs immediately, then range 1's
            # phase-1 streaming (sync ring) overlaps range 0's phase-2
            # multiply+store (scalar ring). Only range 1's fold+MLP remains
            # on the serial boundary before its own phase 2.
            nmm = sum(npts * C // 128 // 512 for _, npts in chunks)

            def p1_range(r, resident):
                m_acc = accs.tile([128, FCMAX], DTX, tag="m_acc")
                ps_s = psum_t.tile([1, 512], DT, tag=f"ps_s{r}")
                mmi = 0
                for ci, (off, npts) in enumerate(chunks):
                    fc = npts * C // 128
                    if (r, ci) in resident:
                        xt = resp.tile([128, fc], DTX, tag=f"res{r}_{ci}")
                        res_tiles[(r, ci)] = xt
                    else:
                        xt = xpool.tile([128, FCMAX], DTX, tag="xt")
                    nc.sync.dma_start(out=xt[:, :fc],
                                      in_=dram_chunk_ap(xs, r, off, npts))
                    if ci == 0:
                        # first chunk is always full-width: init the running
                        # max with a copy (4x bf16 mode) instead of memset+max
                        assert fc == FCMAX
                        nc.vector.tensor_copy(m_acc[:], xt[:, :fc])
                    else:
                        nc.vector.tensor_max(m_acc[:, :fc], m_acc[:, :fc],
                                             xt[:, :fc])
                    for j in range(fc // 512):
                        nc.tensor.matmul(
                            out=ps_s[:], lhsT=ones_bf[:],
                            rhs=xt[:, j * 512:(j + 1) * 512],
                            start=(mmi == 0), stop=(mmi == nmm - 1))
                        mmi += 1
                    yield ci
                yield_state[r] = (m_acc, ps_s)

            def fold_scale(r):
                m_acc, ps_s = yield_state[r]
                # fold sums: [1,512] psum holds (a c) partial sums
                sum_col = small.tile([C, 1], DT, tag=f"sum_col{r}")
                sum_row = small.tile([1, C], DT, tag=f"sum_row{r}")
                nc.vector.reduce_sum(
                    out=sum_row[:],
                    in_=ps_s[:].rearrange("p (a c) -> p c a", c=C),
                    axis=mybir.AxisListType.X)
                sc_ps = psum_w.tile([C, 1], DT, tag="sc")
                nc.tensor.transpose(out=sc_ps[:], in_=sum_row[:],
                                    identity=ident[:1, :1])
                nc.vector.tensor_copy(sum_col[:], sc_ps[:])

                # fold max: in-place binary halvings (2x bf16 tensor_tensor)
                # down to a=16, then a short 1x reduce. Faster than one big
                # 1x reduce, and releases m_acc for the next range sooner.
                fc_cur = FCMAX
                while fc_cur > C * 16:
                    h = fc_cur // 2
                    nc.vector.tensor_max(m_acc[:, :h], m_acc[:, :h],
                                         m_acc[:, h:fc_cur])
                    fc_cur = h
                m64 = small.tile([128, C], DT, tag=f"m64_{r}")
                nc.vector.reduce_max(
                    out=m64[:],
                    in_=m_acc[:, :fc_cur].rearrange("p (a c) -> p c a", c=C),
                    axis=mybir.AxisListType.X)
                mrow_t = psum_t.tile([C, 128], DT, tag="tr")
                nc.tensor.transpose(out=mrow_t[:], in_=m64[:],
                                    identity=ident[:])
                # rhs2 cols: avg, mx
                rhs2 = small.tile([C, 2], DT, tag=f"rhs2_{r}")
                nc.vector.reduce_max(out=rhs2[:, 1:2], in_=mrow_t[:],
                                     axis=mybir.AxisListType.X)
                # avg = (sum - corr) * invc
                nc.vector.tensor_sub(sum_col[:], sum_col[:],
                                     corrt_sb[:, r:r + 1])
                nc.vector.tensor_mul(rhs2[:, 0:1], sum_col[:],
                                     invct_sb[:, r:r + 1])

                # tiny MLP per range: scale = 1 + sigmoid(mlp(avg) + mlp(mx))
                h_ps = psum_w.tile([H, 2], DT, tag="mm")
                nc.tensor.matmul(out=h_ps[:], lhsT=w1t_sb[:], rhs=rhs2[:],
                                 start=True, stop=True)
                h_sb = small.tile([H, 2], DT, tag=f"h_{r}")
                nc.scalar.activation(out=h_sb[:], in_=h_ps[:],
                                     func=mybir.ActivationFunctionType.Relu,
                                     bias=b1c_sb[:])
                z_ps = psum_w.tile([C, 2], DT, tag="mm")
                nc.tensor.matmul(out=z_ps[:], lhsT=w2t_sb[:], rhs=h_sb[:],
                                 start=True, stop=True)
                z_sb = small.tile([C, 2], DT, tag=f"z_{r}")
                nc.vector.tensor_copy(z_sb[:], z_ps[:])
                zsum = small.tile([C, 1], DT, tag=f"zs_{r}")
                nc.vector.tensor_add(zsum[:], z_sb[:, 0:1], z_sb[:, 1:2])
                scol = small.tile([C, 1], DT, tag=f"scol_{r}")
                nc.scalar.activation(out=scol[:], in_=zsum[:],
                                     func=mybir.ActivationFunctionType.Sigmoid,
                                     bias=b2x2_sb[:])
                nc.vector.tensor_scalar_add(scol[:], scol[:], 1.0)
                # broadcast scale column to a [128, C] bf16 tile (all-bf16
                # tensor_mul in phase 2 gets the DVE 2x perf mode)
                row_ps = psum_w.tile([1, C], DT, tag="row")
                nc.tensor.transpose(out=row_ps[:], in_=scol[:],
                                    identity=ident[:C, :C])
                row_sb = small.tile([1, C], DT, tag=f"row_sb{r}")
                nc.vector.tensor_copy(row_sb[:], row_ps[:])
                bcast_ps = psum_w.tile([128, C], DT, tag="bc")
                nc.tensor.matmul(out=bcast_ps[:], lhsT=ones_row[:],
                                 rhs=row_sb[:], start=True, stop=True)
                mult = accs.tile([128, C], DTX, tag=f"mult{r}")
                nc.vector.tensor_copy(mult[:], bcast_ps[:])
                return mult

            def p2_chunk(r, ci, mult):
                off, npts = chunks[ci]
                fa = npts // 128
                mult_bc = mult[:].unsqueeze(1).to_broadcast([128, fa, C])
                if (r, ci) in res_tiles:
                    xt = res_tiles[(r, ci)][:].rearrange(
                        "p (a c) -> p a c", c=C)
                else:
                    t = xpool.tile([128, chunk_tiles * FA, C], DTX, tag="xt")
                    xt = t[:, :fa, :]
                    nc.sync.dma_start(out=xt,
                                      in_=dram_chunk_ap(xs, r, off, npts))
                nc.vector.tensor_mul(xt, xt, mult_bc)
                nc.scalar.dma_start(out=dram_chunk_ap(out, r, off, npts),
                                    in_=xt)

            res_tiles = {}
            yield_state = {}

            for _ in p1_range(0, resident):
                pass
            mult0 = fold_scale(0)
            # range 1 phase-1 loads (sync ring) overlap range 0 phase-2
            # stores (scalar ring); DVE alternates max and mul
            g1 = p1_range(1, resident)
            for ci in g1:
                p2_chunk(0, ci, mult0)
            mult1 = fold_scale(1)
            for ci in range(len(chunks)):
                p2_chunk(1, ci, mult1)

    nc.compile()
    return nc


_CACHE: dict[int, object] = {}


def kernel(x, batch_ids, W1, b1, W2, b2):
    x = np.ascontiguousarray(np.asarray(x, dtype=np.float32))
    batch_ids = np.asarray(batch_ids, dtype=np.int32)
    W1 = np.asarray(W1, dtype=np.float32)
    b1 = np.asarray(b1, dtype=np.float32)
    W2 = np.asarray(W2, dtype=np.float32)
    b2 = np.asarray(b2, dtype=np.float32)

    N = x.shape[0]
    bounds = np.searchsorted(batch_ids, np.arange(B + 1), side="left")
    counts = np.diff(bounds)
    R = max(TP, int(-(-counts.max() // TP)) * TP)

    nc = _CACHE.get(R)
    if nc is None:
        nc = _CACHE[R] = build_nc(R)

    xb = x.astype(NPX)
    xp = np.empty((NCORES, RPC, R, C), NPX)
    corrt = np.zeros((NCORES, C, RPC), np.float32)
    invct = np.zeros((NCORES, C, RPC), np.float32)
    for b in range(B):
        core, r = divmod(b, RPC)
        s, e = int(bounds[b]), int(bounds[b + 1])
        n = e - s
        xp[core, r, :n] = xb[s:e]
        pad = xb[s] if n > 0 else np.zeros(C, NPX)
        xp[core, r, n:] = pad
        # device sums bf16 values in f32 psum; the pad rows contribute
        # (R - n) * f32(bf16(pad)) exactly
        corrt[core, :, r] = np.float64(R - n) * pad.astype(np.float64)
        invct[core, :, r] = 1.0 / max(n, 1)

    w1t = np.ascontiguousarray(W1.T)  # [C, H]
    b1c = np.ascontiguousarray(b1.reshape(H, 1))
    w2t = np.ascontiguousarray(W2.T)  # [H, C]
    b2x2 = np.ascontiguousarray((2.0 * b2).reshape(C, 1))

    in_maps = [
        {
            "xs": xp[core],
            "corrt": np.ascontiguousarray(corrt[core]),
            "invct": np.ascontiguousarray(invct[core]),
            "w1t": w1t,
            "b1c": b1c,
            "w2t": w2t,
            "b2x2": b2x2,
        }
        for core in range(NCORES)
    ]

    res = run_bass_kernel_spmd(nc, in_maps, core_ids=list(range(NCORES)))

    out = np.empty((N, C), np.float32)
    for b in range(B):
        core, r = divmod(b, RPC)
        s, e = int(bounds[b]), int(bounds[b + 1])
        out[s:e] = res.results[core]["out"][r, : e - s].astype(np.float32)
    return out


# revision 16
# speedup vs baseline: 1.0470x; 1.0470x over previous
"""Trainium2 Bass kernel for ChannelAttentionModel (segment avg/max -> tiny MLP ->
sigmoid gate -> per-point scale), SPMD across 8 NeuronCores.

Sharding: batch_ids is sorted with B=16 segments; core k owns batches 2k and
2k+1 (whole batches per device). Each batch range is padded to a fixed R points
by replicating the first row of the batch (max-safe); the extra rows' sum
contribution is subtracted via a host-computed correction term.

x is shipped and processed in bf16 (tolerance is 2e-2; bf16 keeps worst-case
elementwise error well under 2%), halving both host<->device transfer bytes
and on-device HBM traffic. Stats (sum via PE ones-matmul into f32 PSUM, max
via DVE) and the tiny MLP stay in f32. Range 0's chunks (plus range 1's last
chunk) stay resident in SBUF between the stats pass and the scale pass, which
skips their re-read from HBM. Phases are kept sequential on purpose: mixing
HBM reads and writes measurably lowers the sustained DMA rate.
"""

import sys

for _p in ("/opt/trn_rl_repo", "/root/.axon_site/_ro/trn_rl_repo"):
    if _p not in sys.path:
        sys.path.append(_p)

import numpy as np
import ml_dtypes

import concourse.bacc as bacc
import concourse.tile as tile
from concourse import bass, mybir
from concourse.bass_utils import run_bass_kernel_spmd
from concourse.masks import make_identity

NCORES = 8
B = 16
C = 64
H = 32
RPC = 2  # batch ranges per core
TP = 4096  # points per tile
FA = TP // 128  # free-dim point groups per partition (32)
F = FA * C  # free elems per partition per tile (2048)
DT = mybir.dt.float32
DTX = mybir.dt.bfloat16
NPX = ml_dtypes.bfloat16


def build_nc(R: int, chunk_tiles: int = 4, xbufs: int = 3):
    nc = bacc.Bacc("TRN2", target_bir_lowering=False, debug=False,
                   num_devices=NCORES, enable_asserts=False)

    xs = nc.dram_tensor("xs", [RPC, R, C], DTX, kind="ExternalInput")
    corrt = nc.dram_tensor("corrt", [C, RPC], DT, kind="ExternalInput")
    invct = nc.dram_tensor("invct", [C, RPC], DT, kind="ExternalInput")
    w1t = nc.dram_tensor("w1t", [C, H], DT, kind="ExternalInput")
    b1c = nc.dram_tensor("b1c", [H, 1], DT, kind="ExternalInput")
    w2t = nc.dram_tensor("w2t", [H, C], DT, kind="ExternalInput")
    b2x2 = nc.dram_tensor("b2x2", [C, 1], DT, kind="ExternalInput")
    out = nc.dram_tensor("out", [RPC, R, C], DTX, kind="ExternalOutput")

    def dram_chunk_ap(handle, r, off, npts):
        return handle.ap()[r, off:off + npts, :].rearrange(
            "(p a) c -> p (a c)", p=128)

    # chunk each range into large DMA transfers
    chunks = []
    off = 0
    while off < R:
        npts = min(chunk_tiles * TP, R - off)
        chunks.append((off, npts))
        off += npts

    FCMAX = chunk_tiles * F
    HM = FCMAX // 2  # half-width max accumulator

    # Chunks that stay resident in SBUF between phase 1 and phase 2 (skips
    # their phase-2 re-read from HBM): as many of range 0's chunks as fit,
    # plus range 1's last chunk. Budget: <=136 KiB/partition.
    res_budget = 136 * 1024
    resident = set()
    used = 0
    for ci, (off, npts) in enumerate(chunks):
        fc_b = npts * C // 128 * 2  # bf16 bytes per partition
        if used + fc_b > res_budget:
            break
        used += fc_b
        resident.add((0, ci))
    last = len(chunks) - 1
    last_b = chunks[last][1] * C // 128 * 2
    if used + last_b <= res_budget:
        resident.add((1, last))

    with tile.TileContext(nc) as tc:
        with (
            tc.tile_pool(name="const", bufs=1) as const,
            tc.tile_pool(name="resp", bufs=1) as resp,
            tc.tile_pool(name="xpool", bufs=xbufs) as xpool,
            tc.tile_pool(name="accs", bufs=2) as accs,
            tc.tile_pool(name="small", bufs=1) as small,
            tc.tile_pool(name="psum_t", bufs=1, space="PSUM") as psum_t,
            tc.tile_pool(name="psum_w", bufs=1, space="PSUM") as psum_w,
        ):
            # constants
            ident = const.tile([128, 128], DT)
            make_identity(nc, ident[:])
            ones_row = const.tile([1, 128], DT)
            nc.vector.memset(ones_row[:], 1.0)
            ones_bf = const.tile([128, 1], mybir.dt.bfloat16)
            nc.vector.memset(ones_bf[:], 1.0)
            # const loads go on the scalar ring so the first x-chunk loads
            # (sync ring, FIFO) start immediately
            corrt_sb = const.tile([C, RPC], DT)
            nc.scalar.dma_start(out=corrt_sb[:], in_=corrt.ap())
            invct_sb = const.tile([C, RPC], DT)
            nc.scalar.dma_start(out=invct_sb[:], in_=invct.ap())
            w1t_sb = const.tile([C, H], DT)
            nc.scalar.dma_start(out=w1t_sb[:], in_=w1t.ap())
            b1c_sb = const.tile([H, 1], DT)
            nc.scalar.dma_start(out=b1c_sb[:], in_=b1c.ap())
            w2t_sb = const.tile([H, C], DT)
            nc.scalar.dma_start(out=w2t_sb[:], in_=w2t.ap())
            b2x2_sb = const.tile([C, 1], DT)
            nc.scalar.dma_start(out=b2x2_sb[:], in_=b2x2.ap())

            # phase 1: per-range running max (DVE, half-width accumulator so
            # two bufs fit: range r+1's init never waits on range r's fold)
            # + sum (PE ones-matmul into f32 PSUM)
            rhs4 = small.tile([C, 2 * RPC], DT)  # cols: avg0, avg1, mx0, mx1
            res_tiles = {}
            for r in range(RPC):
                m_acc = accs.tile([128, HM], DTX, tag="m_acc")
                ps_s = psum_t.tile([1, 512], DT, tag=f"ps_s{r}")
                nmm = sum(npts * C // 128 // 512 for _, npts in chunks)
                mmi = 0
                for ci, (off, npts) in enumerate(chunks):
                    fc = npts * C // 128
                    if (r, ci) in resident:
                        xt = resp.tile([128, fc], DTX, tag=f"res{r}_{ci}")
                        res_tiles[(r, ci)] = xt
                    else:
                        xt = xpool.tile([128, FCMAX], DTX, tag="xt")
                    nc.sync.dma_start(out=xt[:, :fc],
                                      in_=dram_chunk_ap(xs, r, off, npts))
                    if ci == 0:
                        # first chunk is always full-width: init the running
                        # max with a copy (4x bf16 mode) instead of memset+max
                        assert fc == FCMAX
                        nc.vector.tensor_copy(m_acc[:], xt[:, :HM])
                        nc.vector.tensor_max(m_acc[:], m_acc[:], xt[:, HM:fc])
                    else:
                        nc.vector.tensor_max(m_acc[:, :min(fc, HM)],
                                             m_acc[:, :min(fc, HM)],
                                             xt[:, :min(fc, HM)])
                        if fc > HM:
                            nc.vector.tensor_max(m_acc[:, :fc - HM],
                                                 m_acc[:, :fc - HM],
                                                 xt[:, HM:fc])
                    for j in range(fc // 512):
                        nc.tensor.matmul(
                            out=ps_s[:], lhsT=ones_bf[:],
                            rhs=xt[:, j * 512:(j + 1) * 512],
                            start=(mmi == 0), stop=(mmi == nmm - 1))
                        mmi += 1

                # fold sums: [1,512] psum holds (a c) partial sums
                sum_col = small.tile([C, 1], DT, tag=f"sum_col{r}")
                sum_row = small.tile([1, C], DT, tag=f"sum_row{r}")
                nc.vector.reduce_sum(
                    out=sum_row[:],
                    in_=ps_s[:].rearrange("p (a c) -> p c a", c=C),
                    axis=mybir.AxisListType.X)
                sc_ps = psum_w.tile([C, 1], DT, tag="sc")
                nc.tensor.transpose(out=sc_ps[:], in_=sum_row[:],
                                    identity=ident[:1, :1])
                nc.vector.tensor_copy(sum_col[:], sc_ps[:])

                # fold max: in-place binary halvings (2x bf16 tensor_tensor)
                # down to a=16, then a short 1x reduce
                fc_cur = HM
                while fc_cur > C * 16:
                    h = fc_cur // 2
                    nc.vector.tensor_max(m_acc[:, :h], m_acc[:, :h],
                                         m_acc[:, h:fc_cur])
                    fc_cur = h
                m64 = small.tile([128, C], DT, tag="m64")
                nc.vector.reduce_max(
                    out=m64[:],
                    in_=m_acc[:, :fc_cur].rearrange("p (a c) -> p c a", c=C),
                    axis=mybir.AxisListType.X)
                mrow_t = psum_t.tile([C, 128], DT, tag="tr")
                nc.tensor.transpose(out=mrow_t[:], in_=m64[:], identity=ident[:])
                nc.vector.reduce_max(out=rhs4[:, RPC + r:RPC + r + 1],
                                     in_=mrow_t[:], axis=mybir.AxisListType.X)

                # avg = (sum - corr) * invc
                nc.vector.tensor_sub(sum_col[:], sum_col[:], corrt_sb[:, r:r + 1])
                nc.vector.tensor_mul(rhs4[:, r:r + 1], sum_col[:],
                                     invct_sb[:, r:r + 1])

            # tiny MLP: att = sigmoid(mlp(avg) + mlp(mx)); scale = 1 + att
            h_ps = psum_w.tile([H, 2 * RPC], DT, tag="mm")
            nc.tensor.matmul(out=h_ps[:], lhsT=w1t_sb[:], rhs=rhs4[:],
                             start=True, stop=True)
            h_sb = small.tile([H, 2 * RPC], DT)
            nc.scalar.activation(out=h_sb[:], in_=h_ps[:],
                                 func=mybir.ActivationFunctionType.Relu,
                                 bias=b1c_sb[:])
            z_ps = psum_w.tile([C, 2 * RPC], DT, tag="mm")
            nc.tensor.matmul(out=z_ps[:], lhsT=w2t_sb[:], rhs=h_sb[:],
                             start=True, stop=True)
            z_sb = small.tile([C, 2 * RPC], DT)
            nc.vector.tensor_copy(z_sb[:], z_ps[:])
            zsum = small.tile([C, RPC], DT)
            nc.vector.tensor_add(zsum[:], z_sb[:, 0:RPC], z_sb[:, RPC:2 * RPC])
            scale_t = small.tile([C, RPC], DT)
            nc.scalar.activation(out=scale_t[:], in_=zsum[:],
                                 func=mybir.ActivationFunctionType.Sigmoid,
                                 bias=b2x2_sb[:])
            nc.vector.tensor_scalar_add(scale_t[:], scale_t[:], 1.0)

            # broadcast each range's scale column to [128, C] bf16 tiles
            # (all-bf16 tensor_mul in phase 2 gets the DVE 2x perf mode)
            mults = []
            for r in range(RPC):
                row_ps = psum_w.tile([1, C], DT, tag="row")
                nc.tensor.transpose(out=row_ps[:], in_=scale_t[:, r:r + 1],
                                    identity=ident[:C, :C])
                row_sb = small.tile([1, C], DT, tag=f"row_sb{r}")
                nc.vector.tensor_copy(row_sb[:], row_ps[:])
                bcast_ps = psum_w.tile([128, C], DT, tag="bc")
                nc.tensor.matmul(out=bcast_ps[:], lhsT=ones_row[:], rhs=row_sb[:],
                                 start=True, stop=True)
                mult = accs.tile([128, C], DTX, tag=f"mult{r}")
                nc.vector.tensor_copy(mult[:], bcast_ps[:])
                mults.append(mult)

            # phase 2: out = x * scale[batch]  (load on sync ring, store on
            # scalar ring). Resident chunks need no re-load: multiply in
            # place and store. Interleave streamed (r1) and resident (r0)
            # chunks so the DVE muls recycle the streaming slots steadily.
            order = []
            for ci in range(len(chunks)):
                order.append((1, ci))
                order.append((0, ci))
            for r, ci in order:
                off, npts = chunks[ci]
                fa = npts // 128
                mult_bc = mults[r][:].unsqueeze(1).to_broadcast(
                    [128, fa, C])
                if (r, ci) in res_tiles:
                    xt = res_tiles[(r, ci)][:].rearrange(
                        "p (a c) -> p a c", c=C)
                else:
                    t = xpool.tile([128, chunk_tiles * FA, C], DTX, tag="xt")
                    xt = t[:, :fa, :]
                    nc.sync.dma_start(out=xt,
                                      in_=dram_chunk_ap(xs, r, off, npts))
                nc.vector.tensor_mul(xt, xt, mult_bc)
                nc.scalar.dma_start(out=dram_chunk_ap(out, r, off, npts),
                                    in_=xt)

    nc.compile()
    return nc


_CACHE: dict[int, object] = {}


def kernel(x, batch_ids, W1, b1, W2, b2):
    x = np.ascontiguousarray(np.asarray(x, dtype=np.float32))
    batch_ids = np.asarray(batch_ids, dtype=np.int32)
    W1 = np.asarray(W1, dtype=np.float32)
    b1 = np.asarray(b1, dtype=np.float32)
    W2 = np.asarray(W2, dtype=np.float32)
    b2 = np.asarray(b2, dtype=np.float32)

    N = x.shape[0]
    bounds = np.searchsorted(batch_ids, np.arange(B + 1), side="left")
    counts = np.diff(bounds)
    R = max(TP, int(-(-counts.max() // TP)) * TP)

    nc = _CACHE.get(R)
    if nc is None:
        nc = _CACHE[R] = build_nc(R)

    xb = x.astype(NPX)
    xp = np.empty((NCORES, RPC, R, C), NPX)
    corrt = np.zeros((NCORES, C, RPC), np.float32)
    invct = np.zeros((NCORES, C, RPC), np.float32)
    for b in range(B):
        core, r = divmod(b, RPC)
        s, e = int(bounds[b]), int(bounds[b + 1])
        n = e - s
        xp[core, r, :n] = xb[s:e]
        pad = xb[s] if n > 0 else np.zeros(C, NPX)
        xp[core, r, n:] = pad
        # device sums bf16 values in f32 psum; the pad rows contribute
        # (R - n) * f32(bf16(pad)) exactly
        corrt[core, :, r] = np.float64(R - n) * pad.astype(np.float64)
        invct[core, :, r] = 1.0 / max(n, 1)

    w1t = np.ascontiguousarray(W1.T)  # [C, H]
    b1c = np.ascontiguousarray(b1.reshape(H, 1))
    w2t = np.ascontiguousarray(W2.T)  # [H, C]
    b2x2 = np.ascontiguousarray((2.0 * b2).reshape(C, 1))

    in_maps = [
        {
            "xs": xp[core],
            "corrt": np.ascontiguousarray(corrt[core]),
            "invct": np.ascontiguousarray(invct[core]),
            "w1t": w1t,
            "b1c": b1c,
            "w2t": w2t,
            "b2x2": b2x2,
        }
        for core in range(NCORES)
    ]

    res = run_bass_kernel_spmd(nc, in_maps, core_ids=list(range(NCORES)))

    out = np.empty((N, C), np.float32)
    for b in range(B):
        core, r = divmod(b, RPC)
        s, e = int(bounds[b]), int(bounds[b + 1])
        out[s:e] = res.results[core]["out"][r, : e - s].astype(np.float32)
    return out
